# revision 14
# baseline (speedup 1.0000x reference)
"""Trainium2 Bass kernel for the Attention2 module (sparse attention).

Computation (per batch row b):
    att_h  = h[b] @ W_h.T + b_h                      # [A]
    dot    = tanh(p_att_feats[b] + att_h)            # [L, A]
    scores = dot @ W_a[0]  (+ b_a, dropped: softmax shift-invariant)
    scores = where(mask, -1e8, scores)
    w      = softmax(scores)                         # [L]
    out[b] = w @ att_feats[b]                        # [R]

Sharding: data-parallel over batch B=32 across 8 cores (4 rows/core).

Structure (v32).  The serial backbone is the ACT tanh chain (16 x
~1.5us, 1 elem/lane/cycle, dtype-independent); everything else is
arranged to hide behind it:
  * mask-compaction on host (masked rows never reach the device; pads
    score exactly -sum|W_a| via the tanh-saturation trick).
  * few, large DMAs (v29 spent 40us on 54 serialized ~730ns dma_start
    issues).  p loads issue per-row on the ACT HWDGE ring, everything
    else on the sync ring; the two rings' transfers interleave on the
    16 shared SDMA engines.
  * PE warmup matmuls around phase 0 flip the HAM clock gate
    (1.2 -> 2.4 GHz) before the first score matmul and keep it warm.
  * all four rows' phase-B results accumulate in two shared [128,512]
    PSUM banks at partition 32*b (matmul tile_position from the out
    AP), so the Z-normalization is 2 batched tensor_scalars instead
    of 8, and Z itself is one reduce + one reciprocal over a [128,nch]
    bank the per-row ones-matmuls write at partition 32*b.
  * f split bf16/int8 (int8 dequant on DVE only -- the GPSIMD software
    tensor_scalar path measures ~7.7us per op and poisons concurrent
    DVE ops).
  * host-side prep is layout/dtype only (transposes, gather by mask,
    casts): all arithmetic of the module stays on device.
"""

import sys

import ml_dtypes
import numpy as np

sys.path.insert(0, "/opt/trn_rl_repo")

import concourse.bass as bass  # noqa: E402
import concourse.tile as tile  # noqa: E402
from concourse import bacc, mybir  # noqa: E402
from concourse.bass_utils import run_bass_kernel_spmd  # noqa: E402

N_CORES = 8
B, L, RNN, A = 32, 2048, 1024, 512
BS = B // N_CORES          # 4 batch rows per core
NRC = RNN // 128           # 8 contraction chunks for att_h
NAC = A // 128             # 4 a-chunks (ATT_HID on partitions)
C8 = 512                   # int8 feature columns (tail of RNN)
C16 = RNN - C8             # bf16 feature columns (head of RNN)
N_WARM1 = 75               # PE warmup matmuls before phase 0
N_WARM2 = 40               # gate-pinned warmup matmuls after phase 0

F32 = mybir.dt.float32
FP8 = mybir.dt.float8e4
I8 = mybir.dt.int8
BF16 = mybir.dt.bfloat16
TANH = mybir.ActivationFunctionType.Tanh
EXP = mybir.ActivationFunctionType.Exp

KERNEL_VERSION = 36


def row_cuts(b, nch):
    """f sub-chunk boundaries (lch indices) per row.  Early rows arrive
    long before their phase B -- one chunk.  Late rows are chunked so
    phase-B matmuls start before the full row lands."""
    if b == 0 or b == 1:
        cuts = [0, nch]
    elif b == 2:
        cuts = [0, (nch + 1) // 2, nch]
    else:
        cuts = sorted({0, max(1, nch - 6), max(2, nch - 4), nch - 1, nch})
    return cuts


def build_program(nch, bs=BS, rnn=RNN, a=A):
    lc = nch * 128
    nblob = 4 + bs * nch + 2 + KERNEL_VERSION  # bh | fs | wa(bitcast) | pad
    nc = bacc.Bacc(None, target_bir_lowering=False)
    # p[q, b, ac, l] = p_padded[b, l, ac*128+q]   (A on partitions)
    p = nc.dram_tensor("p", [128, bs, NAC, lc], FP8, kind="ExternalInput")
    # f16[q, b, n, r] = f_padded[b, n*128+q, r] for r < C16 (bf16);
    # f8i holds r >= C16 as int8 with per-position scale (0 for pads)
    f16 = nc.dram_tensor("f16", [128, bs, nch, C16], BF16,
                         kind="ExternalInput")
    f8i = nc.dram_tensor("f8i", [128, bs, nch, C8], I8, kind="ExternalInput")
    # h8[q, rc, b] = h[b, rc*128+q]
    h8 = nc.dram_tensor("h8", [128, NRC, bs], FP8, kind="ExternalInput")
    # wh8[q, ac, rc, j] = W_h[ac*128+j, rc*128+q]  (ac-major for
    # incremental phase 0: tanh ac-chunk k only needs wh8[:, k])
    wh8 = nc.dram_tensor("wh8", [128, NAC, NRC, 128], FP8,
                         kind="ExternalInput")
    # wf32[q, 0:4] = b_h[ac*128+q]; [q, 4+b*nch+n] = scale for pos n*128+q
    # of row b; [q, 4+bs*nch:+2] = W_a bitcast (4 bf16 = 2 f32); pad cols
    # encode KERNEL_VERSION in the shape: the compile cache keys on the
    # HLO signature (names/shapes), not the embedded BIR.
    wf32 = nc.dram_tensor("wf32", [128, nblob], F32, kind="ExternalInput")
    out = nc.dram_tensor("out", [bs, rnn], F32, kind="ExternalOutput")
    fs0 = 4                    # fs column offset in wf32
    wa0 = 4 + bs * nch         # wa column offset in wf32

    with tile.TileContext(nc) as tc:
        with (
            tc.tile_pool(name="singles", bufs=1) as singles,
            tc.tile_pool(name="ppool", bufs=bs) as ppool,
            tc.tile_pool(name="thpool", bufs=bs) as thpool,
            tc.tile_pool(name="fpool", bufs=bs) as fpool,
            tc.tile_pool(name="sm", bufs=4) as smpool,
            tc.tile_pool(name="respool", bufs=1) as respool,
            tc.tile_pool(name="ps_sc", bufs=2, space="PSUM") as ps_sc,
            tc.tile_pool(name="psacc", bufs=1, space="PSUM") as psacc,
            tc.tile_pool(name="pssmall", bufs=1, space="PSUM") as pssmall,
        ):
            # ---- ALL loads go on the sync HWDGE ring in deadline
            # order (EDF): issues (~0.65us each) pipeline far ahead of
            # the ~28us transfer stream, so a single ring sustains full
            # bandwidth AND keeps the ACT ring free for the tanh chain
            # (a dma_start on the ACT ring costs its queue ~0.6-3us).
            h_sb = singles.tile([128, NRC, bs], FP8)
            nc.sync.dma_start(out=h_sb, in_=h8[:, :, :])
            wh_sb = singles.tile([128, NAC, NRC, 128], FP8)
            blob = singles.tile([128, nblob], F32)
            ptiles = []
            for b in range(bs):
                ptiles.append(ppool.tile([128, NAC, lc], FP8, tag="p",
                                         name=f"pt{b}"))
            for ac in range(NAC):
                nc.sync.dma_start(out=wh_sb[:, ac], in_=wh8[:, ac])
            nc.sync.dma_start(out=blob, in_=wf32[:, :])
            nc.sync.dma_start(out=ptiles[0], in_=p[:, 0, :, :])
            nc.sync.dma_start(out=ptiles[1], in_=p[:, 1, :, :])
            wa_ap = blob[:, wa0:wa0 + 2].bitcast(BF16)   # [128, 4]

            # ---- constants; ACT function-table warm while DMAs stream
            ones_sb = singles.tile([128, 1], BF16)
            nc.vector.memset(ones_sb, 1.0)
            warm16 = singles.tile([128, 64], BF16)
            nc.gpsimd.memset(warm16, 0.5)
            attb = singles.tile([128, NAC, bs], F32)
            warm_sb = singles.tile([128, 1], BF16)
            nc.scalar.activation(out=warm_sb, in_=ones_sb, func=TANH)

            # ---- f loads (sync ring), row-major, int8 before bf16 so
            # the dequant chain starts as early as possible per row.
            f16tiles, fitiles, fdtiles = [], [], []
            for b in range(bs):
                f16tiles.append(fpool.tile([128, nch, C16], BF16,
                                           tag="f16", name=f"ft{b}"))
                fitiles.append(fpool.tile([128, nch, C8], I8,
                                          tag="fi", name=f"fi{b}"))
                fdtiles.append(fpool.tile([128, nch, C8], BF16,
                                          tag="fd", name=f"fd{b}"))
            for b in range(bs):
                if b >= 2:
                    nc.sync.dma_start(out=ptiles[b], in_=p[:, b, :, :])
                cuts = row_cuts(b, nch)
                for k in range(len(cuts) - 1):
                    c0, c1 = cuts[k], cuts[k + 1]
                    nc.sync.dma_start(out=fitiles[b][:, c0:c1, :],
                                      in_=f8i[:, b, c0:c1, :])
                    nc.sync.dma_start(out=f16tiles[b][:, c0:c1, :],
                                      in_=f16[:, b, c0:c1, :])

            # ---- PE warmup: dependency-free tiny matmuls from t~0 keep
            # the PE busy so the HAM clock gate opens (1.2 -> 2.4 GHz)
            # before real matmuls run.  Phase 0 is sandwiched between
            # the two warmup bursts: burst 1 spans the wb8 transfer,
            # burst 2 spans row 0's tanh (PE otherwise idle there).
            # one PSUM bank shared by the warmup sink (cols 16:80,
            # partition 0) and the phase-0 accumulator (cols 0:16) --
            # disjoint regions, sequential groups.
            with tc.tile_pool(name="boot", bufs=1, space="PSUM") as boot:
                bt = boot.tile([128, 80], F32, tag="boot")
                wps = bt[0:1, 16:80]
                for _ in range(N_WARM1):
                    nc.tensor.matmul(wps, lhsT=warm16[:, 0:1], rhs=warm16,
                                     start=True, stop=True,
                                     skip_group_check=True)

                # phase 0: attb[:, ac, b] = (W_h @ h[b] + b_h),
                # a-on-partitions; bs as stream dim; per-ac so each
                # attb chunk unblocks its tanh as its W_h chunk lands.
                for ac in range(NAC):
                    for rc in range(NRC):
                        nc.tensor.matmul(
                            bt[:, ac * bs:(ac + 1) * bs],
                            lhsT=wh_sb[:, ac, rc, :],
                            rhs=h_sb[:, rc, :],
                            start=(rc == 0), stop=(rc == NRC - 1),
                            skip_group_check=True)
                    nc.vector.tensor_scalar_add(
                        attb[:, ac, :], bt[:, ac * bs:(ac + 1) * bs],
                        blob[:, ac:ac + 1])
                    if ac == 0:
                        gate = singles.tile([128, 1], BF16)
                        nc.vector.tensor_scalar_mul(gate, ones_sb,
                                                    attb[:, 0, 0:1])

                # gate-pinned warmup: rhs is written on DVE after the
                # first attb add, so the scheduler cannot hoist these
                # ahead of phase 0 -- they fill the PE idle window
                # between phase 0 and the first score matmuls (idle
                # re-throttles the HAM clock gate).
                for _ in range(N_WARM2):
                    nc.tensor.matmul(wps[:, 0:1], lhsT=warm16[:, 0:1],
                                     rhs=gate, start=True, stop=True,
                                     skip_group_check=True)

            # shared result/Z banks: row b lives at partition 64*(b%2)
            # of bank-pair b//2 (matmul tile_position inferred from the
            # out AP; base partitions are limited to {0, 32, 64}).
            z_ps = pssmall.tile([128, 2, nch], F32, tag="zps")
            rps = [[psacc.tile([128, 512], F32, tag=f"r{j}{hh}",
                               name=f"r{j}{hh}")
                    for hh in range(2)] for j in range(2)]

            # ---- pass 1 (phase A for every row)
            w_sbs = []
            for b in range(bs):
                ptile = ptiles[b]
                # tanh with fused per-partition bias (fp8 in, bf16 out)
                th = thpool.tile([128, NAC, lc], BF16, tag="th",
                                 name=f"th{b}")
                for ac in range(NAC):
                    nc.scalar.activation(
                        out=th[:, ac, :], in_=ptile[:, ac, :],
                        func=TANH, bias=attb[:, ac, b:b + 1])
                # scores: lch-outer so each PSUM column's accumulation
                # group is issued contiguously (interleaved groups in one
                # bank corrupt accumulation on HW)
                sc_ps = ps_sc.tile([128, nch], F32, tag="sc", name=f"sc{b}")
                for lch in range(nch):
                    for ac in range(NAC):
                        nc.tensor.matmul(
                            sc_ps[:, lch:lch + 1],
                            lhsT=th[:, ac, lch * 128:(lch + 1) * 128],
                            rhs=wa_ap[:, ac:ac + 1],
                            start=(ac == 0), stop=(ac == NAC - 1))

                # softmax weights (no max subtraction needed: |scores| <=
                # sum|W_a| ~ 11.3, exp cannot overflow f32).  Z partial:
                # ones-matmul row-sum into partition 32*b of the shared
                # Z bank.
                w_sb = smpool.tile([128, nch], BF16, tag="w", name=f"w{b}")
                nc.scalar.activation(out=w_sb, in_=sc_ps, func=EXP)
                zrow = z_ps[64 * (b % 2):64 * (b % 2) + 1, b // 2, :]
                nc.tensor.matmul(zrow, lhsT=ones_sb, rhs=w_sb,
                                 start=True, stop=True,
                                 skip_group_check=True)
                w_sbs.append(w_sb)

            # ---- pass 2 (phase B): row b accumulates w_b @ f_b into
            # partition 32*b of the two shared half-banks.
            def emit_dq(b):
                # dequantize the int8 half on DVE: per-position scale is
                # a per-partition scalar per lch
                fi, fd = fitiles[b], fdtiles[b]
                for lch in range(nch):
                    nc.vector.tensor_scalar_mul(
                        fd[:, lch, :], fi[:, lch, :],
                        blob[:, fs0 + b * nch + lch:fs0 + b * nch + lch + 1])

            def finalize_pair(j):
                # Z and normalization, batched across the pair's rows:
                # one reduce, one reciprocal, two scales, two stores.
                # Garbage at partitions not in {0, 64} is never read.
                zsum = smpool.tile([128, 1], F32, tag="zsum", name=f"zs{j}")
                nc.vector.tensor_reduce(zsum, z_ps[:, j, :],
                                        mybir.AxisListType.X,
                                        mybir.AluOpType.add)
                zinv = smpool.tile([128, 1], F32, tag="zinv", name=f"zi{j}")
                nc.vector.reciprocal(zinv, zsum)
                res = respool.tile([128, rnn], F32, tag=f"res{j}")
                for hh in range(2):
                    nc.vector.tensor_scalar_mul(
                        res[:, hh * 512:(hh + 1) * 512], rps[j][hh], zinv)
                for k in range(2):
                    ring = nc.sync if k == 0 else nc.scalar
                    ring.dma_start(out=out[2 * j + k:2 * j + k + 1, :],
                                   in_=res[64 * k:64 * k + 1, :])

            emit_dq(0)
            emit_dq(1)
            for b in range(bs):
                w_sb = w_sbs[b]
                # bf16 half first (no dequant dependency) except the last
                # row, whose int8 chunks land first in the DMA stream.
                halves = ((0, f16tiles[b]), (1, fdtiles[b]))
                if b == bs - 1:
                    halves = (halves[1], halves[0])
                first = True
                for hh, tile_src in halves:
                    dst = rps[b // 2][hh][64 * (b % 2):64 * (b % 2) + 1, :]
                    for lch in range(nch):
                        nc.tensor.matmul(
                            dst, lhsT=w_sb[:, lch:lch + 1],
                            rhs=tile_src[:, lch, :],
                            start=(lch == 0), stop=(lch == nch - 1),
                            skip_group_check=True)
                    if first and b + 2 < bs:
                        emit_dq(b + 2)
                        first = False
                if b % 2 == 1:
                    finalize_pair(b // 2)
    nc.finalize()
    return nc


_PROGS = {}


def _get_program(nch):
    if nch not in _PROGS:
        _PROGS[nch] = build_program(nch)
    return _PROGS[nch]


def make_in_maps(h, att_feats, p_att_feats, mask, W_h, b_h, W_a):
    h = np.asarray(h, dtype=np.float32)
    att_feats = np.asarray(att_feats, dtype=np.float32)
    p_att_feats = np.asarray(p_att_feats, dtype=np.float32)
    mask = np.asarray(mask)
    W_h = np.asarray(W_h, np.float32)
    b_h = np.asarray(b_h, np.float32).reshape(A)
    wa = np.asarray(W_a, np.float32).reshape(A)

    keep = ~mask                                   # [B, L] kept positions
    cnts = keep.sum(axis=1)
    nch = max(1, -(-int(cnts.max()) // 128))
    lc = nch * 128

    # padding p-row: tanh saturates to -sign(wa) => score = -sum|wa|
    p_pad = np.where(wa >= 0.0, -20.0, 20.0).astype(ml_dtypes.float8_e4m3)

    p2 = np.empty((B, 128, NAC, lc), dtype=ml_dtypes.float8_e4m3)
    f16h = np.zeros((B, 128, nch, C16), dtype=ml_dtypes.bfloat16)
    f8ih = np.zeros((B, 128, nch, C8), dtype=np.int8)
    fsc = np.zeros((B, lc), dtype=np.float32)
    for b in range(B):
        idx = np.flatnonzero(keep[b])
        n = idx.size
        pb = np.empty((lc, A), dtype=ml_dtypes.float8_e4m3)
        pb[:n] = p_att_feats[b, idx]
        pb[n:] = p_pad
        # [lc, A] -> [128(q), NAC, lc]
        p2[b] = pb.reshape(lc, NAC, 128).transpose(2, 1, 0)
        fb = att_feats[b, idx]                          # [n, RNN]
        lo = np.zeros((lc, C16), dtype=ml_dtypes.bfloat16)
        lo[:n] = fb[:, :C16].astype(ml_dtypes.bfloat16)
        f16h[b] = lo.reshape(nch, 128, C16).transpose(1, 0, 2)
        # int8 half with per-position scale (exact 0 for pads)
        hi = fb[:, C16:]
        amax = np.abs(hi).max(axis=1)
        scale = amax / 127.0
        q = np.rint(hi / scale[:, None]).clip(-127, 127).astype(np.int8)
        qpad = np.zeros((lc, C8), dtype=np.int8)
        qpad[:n] = q
        f8ih[b] = qpad.reshape(nch, 128, C8).transpose(1, 0, 2)
        fsc[b, :n] = scale

    # wh8[q, ac, rc, j] = W_h[ac*128+j, rc*128+q]
    wh8 = np.ascontiguousarray(
        W_h.reshape(NAC, 128, NRC, 128).transpose(3, 0, 2, 1)).astype(
            ml_dtypes.float8_e4m3)                      # [128, NAC, NRC, 128]
    bh2 = np.ascontiguousarray(b_h.reshape(NAC, 128).T)  # [128, NAC] f32
    wa2 = np.ascontiguousarray(wa.reshape(NAC, 128).T).astype(
        ml_dtypes.bfloat16)                             # [128, NAC]
    wa_f32view = np.ascontiguousarray(wa2).view(np.float32)  # [128, 2]

    nblob = 4 + BS * nch + 2 + KERNEL_VERSION
    in_maps = []
    for c in range(N_CORES):
        s = slice(c * BS, (c + 1) * BS)
        h2c = np.ascontiguousarray(
            h[s].reshape(BS, NRC, 128).transpose(2, 1, 0)).astype(
                ml_dtypes.float8_e4m3)                  # [128, NRC, BS]
        # fs[q, b, n]: scale for position l = n*128+q of row b
        fsc_c = np.ascontiguousarray(
            fsc[s].reshape(BS, nch, 128).transpose(2, 0, 1))
        wf32 = np.zeros((128, nblob), dtype=np.float32)
        wf32[:, 0:4] = bh2
        wf32[:, 4:4 + BS * nch] = fsc_c.reshape(128, BS * nch)
        wf32[:, 4 + BS * nch:4 + BS * nch + 2] = wa_f32view
        in_maps.append({
            "p": np.ascontiguousarray(p2[s].transpose(1, 0, 2, 3)),
            "f16": np.ascontiguousarray(f16h[s].transpose(1, 0, 2, 3)),
            "f8i": np.ascontiguousarray(f8ih[s].transpose(1, 0, 2, 3)),
            "h8": h2c,
            "wh8": wh8,
            "wf32": wf32,
        })
    return in_maps, nch


def run_sharded(inputs, trace=False, **kwargs):
    in_maps, nch = make_in_maps(
        inputs["h"], inputs["att_feats"], inputs["p_att_feats"],
        inputs["mask"], inputs["W_h"], inputs["b_h"], inputs["W_a"])
    nc = _get_program(nch)
    return run_bass_kernel_spmd(nc, in_maps, core_ids=list(range(N_CORES)),
                                trace=trace, **kwargs)


def kernel(h, att_feats, p_att_feats, mask, W_h, b_h, W_a, b_a):
    res = run_sharded({
        "h": h, "att_feats": att_feats, "p_att_feats": p_att_feats,
        "mask": mask, "W_h": W_h, "b_h": b_h, "W_a": W_a, "b_a": b_a})
    return np.concatenate([res.results[c]["out"] for c in range(N_CORES)],
                          axis=0).astype(np.float32)


# revision 17
# speedup vs baseline: 1.0291x; 1.0291x over previous
"""Trainium2 Bass kernel for the Attention2 module (sparse attention).

Computation (per batch row b):
    att_h  = h[b] @ W_h.T + b_h                      # [A]
    dot    = tanh(p_att_feats[b] + att_h)            # [L, A]
    scores = dot @ W_a[0]  (+ b_a, dropped: softmax shift-invariant)
    scores = where(mask, -1e8, scores)
    w      = softmax(scores)                         # [L]
    out[b] = w @ att_feats[b]                        # [R]

Sharding: data-parallel over batch B=32 across 8 cores (4 rows/core).

Structure (v32).  The serial backbone is the ACT tanh chain (16 x
~1.5us, 1 elem/lane/cycle, dtype-independent); everything else is
arranged to hide behind it:
  * mask-compaction on host (masked rows never reach the device; pads
    score exactly -sum|W_a| via the tanh-saturation trick).
  * few, large DMAs (v29 spent 40us on 54 serialized ~730ns dma_start
    issues).  p loads issue per-row on the ACT HWDGE ring, everything
    else on the sync ring; the two rings' transfers interleave on the
    16 shared SDMA engines.
  * PE warmup matmuls around phase 0 flip the HAM clock gate
    (1.2 -> 2.4 GHz) before the first score matmul and keep it warm.
  * all four rows' phase-B results accumulate in two shared [128,512]
    PSUM banks at partition 32*b (matmul tile_position from the out
    AP), so the Z-normalization is 2 batched tensor_scalars instead
    of 8, and Z itself is one reduce + one reciprocal over a [128,nch]
    bank the per-row ones-matmuls write at partition 32*b.
  * f split bf16/int8 (int8 dequant on DVE only -- the GPSIMD software
    tensor_scalar path measures ~7.7us per op and poisons concurrent
    DVE ops).
  * host-side prep is layout/dtype only (transposes, gather by mask,
    casts): all arithmetic of the module stays on device.
"""

import sys

import ml_dtypes
import numpy as np

sys.path.insert(0, "/opt/trn_rl_repo")

import concourse.bass as bass  # noqa: E402
import concourse.tile as tile  # noqa: E402
from concourse import bacc, mybir  # noqa: E402
from concourse.bass_utils import run_bass_kernel_spmd  # noqa: E402

N_CORES = 8
B, L, RNN, A = 32, 2048, 1024, 512
BS = B // N_CORES          # 4 batch rows per core
NRC = RNN // 128           # 8 contraction chunks for att_h
NAC = A // 128             # 4 a-chunks (ATT_HID on partitions)
C8 = 512                   # int8 feature columns (tail of RNN)
C16 = RNN - C8             # bf16 feature columns (head of RNN)
N_WARM1 = 92               # PE warmup matmuls before phase 0
N_WARM2 = 40               # gate-pinned warmup matmuls after phase 0

F32 = mybir.dt.float32
FP8 = mybir.dt.float8e4
I8 = mybir.dt.int8
BF16 = mybir.dt.bfloat16
TANH = mybir.ActivationFunctionType.Tanh
EXP = mybir.ActivationFunctionType.Exp

KERNEL_VERSION = 37


def row_cuts(b, nch):
    """f sub-chunk boundaries (lch indices) per row.  Early rows arrive
    long before their phase B -- one chunk.  Late rows are chunked so
    phase-B matmuls start before the full row lands."""
    if b == 0 or b == 1:
        cuts = [0, nch]
    elif b == 2:
        cuts = [0, (nch + 1) // 2, nch]
    else:
        cuts = sorted({0, max(1, nch - 6), max(2, nch - 4), nch - 1, nch})
    return cuts


def build_program(nch, bs=BS, rnn=RNN, a=A):
    lc = nch * 128
    nblob = 4 + bs * nch + 2 + KERNEL_VERSION  # bh | fs | wa(bitcast) | pad
    nc = bacc.Bacc(None, target_bir_lowering=False)
    # p[q, b, ac, l] = p_padded[b, l, ac*128+q]   (A on partitions)
    p = nc.dram_tensor("p", [128, bs, NAC, lc], FP8, kind="ExternalInput")
    # f16[q, b, n, r] = f_padded[b, n*128+q, r] for r < C16 (bf16);
    # f8i holds r >= C16 as int8 with per-position scale (0 for pads)
    f16 = nc.dram_tensor("f16", [128, bs, nch, C16], BF16,
                         kind="ExternalInput")
    f8i = nc.dram_tensor("f8i", [128, bs, nch, C8], I8, kind="ExternalInput")
    # ALL small weights in ONE fp8 blob with large contiguous
    # per-partition descriptors -- a transfer whose per-partition run is
    # tiny (e.g. 32B) head-of-line-blocks every SDMA engine for ~6us.
    #   cols [0:4096)            wh[q, ac*1024+rc*128+j] = W_h[ac*128+j,
    #                            rc*128+q]   (ac-major)
    #   cols [4096:4128)         h[q, rc*bs+b] = h[b, rc*128+q]
    #   cols [4128:4128+4*nblob) f32 blob bitcast: b_h | fs | W_a | pad
    #                            (pad encodes KERNEL_VERSION: the compile
    #                            cache keys on shapes, not embedded BIR)
    wball = nc.dram_tensor("wball", [128, 4096 + NRC * bs + 4 * nblob],
                           FP8, kind="ExternalInput")
    out = nc.dram_tensor("out", [bs, rnn], F32, kind="ExternalOutput")
    fs0 = 4                    # fs column offset in wf32
    wa0 = 4 + bs * nch         # wa column offset in wf32

    with tile.TileContext(nc) as tc:
        with (
            tc.tile_pool(name="singles", bufs=1) as singles,
            tc.tile_pool(name="ppool", bufs=bs) as ppool,
            tc.tile_pool(name="thpool", bufs=bs) as thpool,
            tc.tile_pool(name="fpool", bufs=bs) as fpool,
            tc.tile_pool(name="sm", bufs=4) as smpool,
            tc.tile_pool(name="respool", bufs=1) as respool,
            tc.tile_pool(name="ps_sc", bufs=2, space="PSUM") as ps_sc,
            tc.tile_pool(name="psacc", bufs=1, space="PSUM") as psacc,
            tc.tile_pool(name="pssmall", bufs=1, space="PSUM") as pssmall,
        ):
            # ---- ALL loads go on the sync HWDGE ring in deadline
            # order (EDF): issues (~0.65us each) pipeline far ahead of
            # the ~28us transfer stream, so a single ring sustains full
            # bandwidth AND keeps the ACT ring free for the tanh chain
            # (a dma_start on the ACT ring costs its queue ~0.6-3us).
            wball_sb = singles.tile([128, 4096 + NRC * bs + 4 * nblob], FP8)
            nc.sync.dma_start(out=wball_sb, in_=wball[:, :])
            ptiles = []
            for b in range(bs):
                ptiles.append(ppool.tile([128, NAC, lc], FP8, tag="p",
                                         name=f"pt{b}"))
            nc.sync.dma_start(out=ptiles[0][:, 0:2, :], in_=p[:, 0, 0:2, :])
            nc.sync.dma_start(out=ptiles[0][:, 2:4, :], in_=p[:, 0, 2:4, :])
            nc.sync.dma_start(out=ptiles[1], in_=p[:, 1, :, :])
            h0 = 4096
            blob = wball_sb[:, h0 + NRC * bs:].bitcast(F32)   # [128, nblob]
            wa_ap = wball_sb[:, h0 + NRC * bs + 4 * wa0:
                             h0 + NRC * bs + 4 * wa0 + 8].bitcast(BF16)

            # ---- constants; ACT function-table warm while DMAs stream
            ones_sb = singles.tile([128, 1], BF16)
            nc.vector.memset(ones_sb, 1.0)
            warm16 = singles.tile([128, 64], BF16)
            nc.gpsimd.memset(warm16, 0.5)
            attb = singles.tile([128, NAC, bs], F32)
            warm_sb = singles.tile([128, 1], BF16)
            nc.scalar.activation(out=warm_sb, in_=ones_sb, func=TANH)

            # ---- f loads (sync ring), row-major, int8 before bf16 so
            # the dequant chain starts as early as possible per row.
            f16tiles, fitiles, fdtiles = [], [], []
            for b in range(bs):
                f16tiles.append(fpool.tile([128, nch, C16], BF16,
                                           tag="f16", name=f"ft{b}"))
                fitiles.append(fpool.tile([128, nch, C8], I8,
                                          tag="fi", name=f"fi{b}"))
                fdtiles.append(fpool.tile([128, nch, C8], BF16,
                                          tag="fd", name=f"fd{b}"))
            for b in range(bs):
                if b >= 2:
                    nc.sync.dma_start(out=ptiles[b], in_=p[:, b, :, :])
                cuts = row_cuts(b, nch)
                for k in range(len(cuts) - 1):
                    c0, c1 = cuts[k], cuts[k + 1]
                    nc.sync.dma_start(out=fitiles[b][:, c0:c1, :],
                                      in_=f8i[:, b, c0:c1, :])
                    nc.sync.dma_start(out=f16tiles[b][:, c0:c1, :],
                                      in_=f16[:, b, c0:c1, :])

            # ---- PE warmup: dependency-free tiny matmuls from t~0 keep
            # the PE busy so the HAM clock gate opens (1.2 -> 2.4 GHz)
            # before real matmuls run.  Phase 0 is sandwiched between
            # the two warmup bursts: burst 1 spans the wb8 transfer,
            # burst 2 spans row 0's tanh (PE otherwise idle there).
            # one PSUM bank shared by the warmup sink (cols 16:80,
            # partition 0) and the phase-0 accumulator (cols 0:16) --
            # disjoint regions, sequential groups.
            with tc.tile_pool(name="boot", bufs=1, space="PSUM") as boot:
                bt = boot.tile([128, 80], F32, tag="boot")
                wps = bt[0:1, 16:80]
                for _ in range(N_WARM1):
                    nc.tensor.matmul(wps, lhsT=warm16[:, 0:1], rhs=warm16,
                                     start=True, stop=True,
                                     skip_group_check=True)

                # phase 0: attb[:, ac, b] = (W_h @ h[b] + b_h),
                # a-on-partitions; bs as stream dim; per-ac so each
                # attb chunk unblocks its tanh as its W_h chunk lands.
                for ac in range(NAC):
                    for rc in range(NRC):
                        nc.tensor.matmul(
                            bt[:, ac * bs:(ac + 1) * bs],
                            lhsT=wball_sb[:, ac * 1024 + rc * 128:
                                          ac * 1024 + rc * 128 + 128],
                            rhs=wball_sb[:, h0 + rc * bs:
                                         h0 + (rc + 1) * bs],
                            start=(rc == 0), stop=(rc == NRC - 1),
                            skip_group_check=True)
                    nc.vector.tensor_scalar_add(
                        attb[:, ac, :], bt[:, ac * bs:(ac + 1) * bs],
                        blob[:, ac:ac + 1])
                    if ac == 0:
                        gate = singles.tile([128, 1], BF16)
                        nc.vector.tensor_scalar_mul(gate, ones_sb,
                                                    attb[:, 0, 0:1])

                # gate-pinned warmup: rhs is written on DVE after the
                # first attb add, so the scheduler cannot hoist these
                # ahead of phase 0 -- they fill the PE idle window
                # between phase 0 and the first score matmuls (idle
                # re-throttles the HAM clock gate).
                for _ in range(N_WARM2):
                    nc.tensor.matmul(wps[:, 0:1], lhsT=warm16[:, 0:1],
                                     rhs=gate, start=True, stop=True,
                                     skip_group_check=True)

            # shared result/Z banks: row b lives at partition 64*(b%2)
            # of bank-pair b//2 (matmul tile_position inferred from the
            # out AP; base partitions are limited to {0, 32, 64}).
            z_ps = pssmall.tile([128, 2, nch], F32, tag="zps")
            rps = [[psacc.tile([128, 512], F32, tag=f"r{j}{hh}",
                               name=f"r{j}{hh}")
                    for hh in range(2)] for j in range(2)]

            # ---- pass 1 (phase A for every row)
            w_sbs = []
            for b in range(bs):
                ptile = ptiles[b]
                # tanh with fused per-partition bias (fp8 in, bf16 out)
                th = thpool.tile([128, NAC, lc], BF16, tag="th",
                                 name=f"th{b}")
                for ac in range(NAC):
                    nc.scalar.activation(
                        out=th[:, ac, :], in_=ptile[:, ac, :],
                        func=TANH, bias=attb[:, ac, b:b + 1])
                # scores: lch-outer so each PSUM column's accumulation
                # group is issued contiguously (interleaved groups in one
                # bank corrupt accumulation on HW)
                sc_ps = ps_sc.tile([128, nch], F32, tag="sc", name=f"sc{b}")
                for lch in range(nch):
                    for ac in range(NAC):
                        nc.tensor.matmul(
                            sc_ps[:, lch:lch + 1],
                            lhsT=th[:, ac, lch * 128:(lch + 1) * 128],
                            rhs=wa_ap[:, ac:ac + 1],
                            start=(ac == 0), stop=(ac == NAC - 1))

                # softmax weights (no max subtraction needed: |scores| <=
                # sum|W_a| ~ 11.3, exp cannot overflow f32).  Z partial:
                # ones-matmul row-sum into partition 32*b of the shared
                # Z bank.
                w_sb = smpool.tile([128, nch], BF16, tag="w", name=f"w{b}")
                nc.scalar.activation(out=w_sb, in_=sc_ps, func=EXP)
                zrow = z_ps[64 * (b % 2):64 * (b % 2) + 1, b // 2, :]
                nc.tensor.matmul(zrow, lhsT=ones_sb, rhs=w_sb,
                                 start=True, stop=True,
                                 skip_group_check=True)
                w_sbs.append(w_sb)

            # ---- pass 2 (phase B): row b accumulates w_b @ f_b into
            # partition 32*b of the two shared half-banks.
            def emit_dq(b):
                # dequantize the int8 half on DVE: per-position scale is
                # a per-partition scalar per lch
                fi, fd = fitiles[b], fdtiles[b]
                for lch in range(nch):
                    nc.vector.tensor_scalar_mul(
                        fd[:, lch, :], fi[:, lch, :],
                        blob[:, fs0 + b * nch + lch:fs0 + b * nch + lch + 1])

            def finalize_pair(j):
                # Z and normalization, batched across the pair's rows:
                # one reduce, one reciprocal, two scales, two stores.
                # Garbage at partitions not in {0, 64} is never read.
                zsum = smpool.tile([128, 1], F32, tag="zsum", name=f"zs{j}")
                nc.vector.tensor_reduce(zsum, z_ps[:, j, :],
                                        mybir.AxisListType.X,
                                        mybir.AluOpType.add)
                zinv = smpool.tile([128, 1], F32, tag="zinv", name=f"zi{j}")
                nc.vector.reciprocal(zinv, zsum)
                res = respool.tile([128, rnn], F32, tag=f"res{j}")
                for hh in range(2):
                    nc.vector.tensor_scalar_mul(
                        res[:, hh * 512:(hh + 1) * 512], rps[j][hh], zinv)
                for k in range(2):
                    ring = nc.sync if k == 0 else nc.scalar
                    ring.dma_start(out=out[2 * j + k:2 * j + k + 1, :],
                                   in_=res[64 * k:64 * k + 1, :])

            emit_dq(0)
            emit_dq(1)
            for b in range(bs):
                w_sb = w_sbs[b]
                # bf16 half first (no dequant dependency) except the last
                # row, whose int8 chunks land first in the DMA stream.
                halves = ((0, f16tiles[b]), (1, fdtiles[b]))
                if b == bs - 1:
                    halves = (halves[1], halves[0])
                first = True
                for hh, tile_src in halves:
                    dst = rps[b // 2][hh][64 * (b % 2):64 * (b % 2) + 1, :]
                    for lch in range(nch):
                        nc.tensor.matmul(
                            dst, lhsT=w_sb[:, lch:lch + 1],
                            rhs=tile_src[:, lch, :],
                            start=(lch == 0), stop=(lch == nch - 1),
                            skip_group_check=True)
                    if first and b + 2 < bs:
                        emit_dq(b + 2)
                        first = False
                if b % 2 == 1:
                    finalize_pair(b // 2)
    nc.finalize()
    return nc


_PROGS = {}


def _get_program(nch):
    if nch not in _PROGS:
        _PROGS[nch] = build_program(nch)
    return _PROGS[nch]


def make_in_maps(h, att_feats, p_att_feats, mask, W_h, b_h, W_a):
    h = np.asarray(h, dtype=np.float32)
    att_feats = np.asarray(att_feats, dtype=np.float32)
    p_att_feats = np.asarray(p_att_feats, dtype=np.float32)
    mask = np.asarray(mask)
    W_h = np.asarray(W_h, np.float32)
    b_h = np.asarray(b_h, np.float32).reshape(A)
    wa = np.asarray(W_a, np.float32).reshape(A)

    keep = ~mask                                   # [B, L] kept positions
    cnts = keep.sum(axis=1)
    nch = max(1, -(-int(cnts.max()) // 128))
    lc = nch * 128

    # padding p-row: tanh saturates to -sign(wa) => score = -sum|wa|
    p_pad = np.where(wa >= 0.0, -20.0, 20.0).astype(ml_dtypes.float8_e4m3)

    p2 = np.empty((B, 128, NAC, lc), dtype=ml_dtypes.float8_e4m3)
    f16h = np.zeros((B, 128, nch, C16), dtype=ml_dtypes.bfloat16)
    f8ih = np.zeros((B, 128, nch, C8), dtype=np.int8)
    fsc = np.zeros((B, lc), dtype=np.float32)
    for b in range(B):
        idx = np.flatnonzero(keep[b])
        n = idx.size
        pb = np.empty((lc, A), dtype=ml_dtypes.float8_e4m3)
        pb[:n] = p_att_feats[b, idx]
        pb[n:] = p_pad
        # [lc, A] -> [128(q), NAC, lc]
        p2[b] = pb.reshape(lc, NAC, 128).transpose(2, 1, 0)
        fb = att_feats[b, idx]                          # [n, RNN]
        lo = np.zeros((lc, C16), dtype=ml_dtypes.bfloat16)
        lo[:n] = fb[:, :C16].astype(ml_dtypes.bfloat16)
        f16h[b] = lo.reshape(nch, 128, C16).transpose(1, 0, 2)
        # int8 half with per-position scale (exact 0 for pads)
        hi = fb[:, C16:]
        amax = np.abs(hi).max(axis=1)
        scale = amax / 127.0
        q = np.rint(hi / scale[:, None]).clip(-127, 127).astype(np.int8)
        qpad = np.zeros((lc, C8), dtype=np.int8)
        qpad[:n] = q
        f8ih[b] = qpad.reshape(nch, 128, C8).transpose(1, 0, 2)
        fsc[b, :n] = scale

    # wh8[q, ac*1024+rc*128+j] = W_h[ac*128+j, rc*128+q]
    wh8 = np.ascontiguousarray(
        W_h.reshape(NAC, 128, NRC, 128).transpose(3, 0, 2, 1)).astype(
            ml_dtypes.float8_e4m3).reshape(128, 4096)
    bh2 = np.ascontiguousarray(b_h.reshape(NAC, 128).T)  # [128, NAC] f32
    wa2 = np.ascontiguousarray(wa.reshape(NAC, 128).T).astype(
        ml_dtypes.bfloat16)                             # [128, NAC]
    wa_f32view = np.ascontiguousarray(wa2).view(np.float32)  # [128, 2]

    nblob = 4 + BS * nch + 2 + KERNEL_VERSION
    in_maps = []
    for c in range(N_CORES):
        s = slice(c * BS, (c + 1) * BS)
        h2c = np.ascontiguousarray(
            h[s].reshape(BS, NRC, 128).transpose(2, 1, 0)).astype(
                ml_dtypes.float8_e4m3).reshape(128, NRC * BS)
        # fs[q, b, n]: scale for position l = n*128+q of row b
        fsc_c = np.ascontiguousarray(
            fsc[s].reshape(BS, nch, 128).transpose(2, 0, 1))
        wf32 = np.zeros((128, nblob), dtype=np.float32)
        wf32[:, 0:4] = bh2
        wf32[:, 4:4 + BS * nch] = fsc_c.reshape(128, BS * nch)
        wf32[:, 4 + BS * nch:4 + BS * nch + 2] = wa_f32view
        wball = np.empty((128, 4096 + NRC * BS + 4 * nblob),
                         dtype=ml_dtypes.float8_e4m3)
        wball[:, 0:4096] = wh8
        wball[:, 4096:4096 + NRC * BS] = h2c
        wball[:, 4096 + NRC * BS:] = wf32.view(np.uint8).view(
            ml_dtypes.float8_e4m3)
        in_maps.append({
            "p": np.ascontiguousarray(p2[s].transpose(1, 0, 2, 3)),
            "f16": np.ascontiguousarray(f16h[s].transpose(1, 0, 2, 3)),
            "f8i": np.ascontiguousarray(f8ih[s].transpose(1, 0, 2, 3)),
            "wball": wball,
        })
    return in_maps, nch


def run_sharded(inputs, trace=False, **kwargs):
    in_maps, nch = make_in_maps(
        inputs["h"], inputs["att_feats"], inputs["p_att_feats"],
        inputs["mask"], inputs["W_h"], inputs["b_h"], inputs["W_a"])
    nc = _get_program(nch)
    return run_bass_kernel_spmd(nc, in_maps, core_ids=list(range(N_CORES)),
                                trace=trace, **kwargs)


def kernel(h, att_feats, p_att_feats, mask, W_h, b_h, W_a, b_a):
    res = run_sharded({
        "h": h, "att_feats": att_feats, "p_att_feats": p_att_feats,
        "mask": mask, "W_h": W_h, "b_h": b_h, "W_a": W_a, "b_a": b_a})
    return np.concatenate([res.results[c]["out"] for c in range(N_CORES)],
                          axis=0).astype(np.float32)


# revision 18
# speedup vs baseline: 1.0598x; 1.0298x over previous
"""Trainium2 Bass kernel for the Attention2 module (sparse attention).

Computation (per batch row b):
    att_h  = h[b] @ W_h.T + b_h                      # [A]
    dot    = tanh(p_att_feats[b] + att_h)            # [L, A]
    scores = dot @ W_a[0]  (+ b_a, dropped: softmax shift-invariant)
    scores = where(mask, -1e8, scores)
    w      = softmax(scores)                         # [L]
    out[b] = w @ att_feats[b]                        # [R]

Sharding: data-parallel over batch B=32 across 8 cores (4 rows/core).

Structure (v32).  The serial backbone is the ACT tanh chain (16 x
~1.5us, 1 elem/lane/cycle, dtype-independent); everything else is
arranged to hide behind it:
  * mask-compaction on host (masked rows never reach the device; pads
    score exactly -sum|W_a| via the tanh-saturation trick).
  * few, large DMAs (v29 spent 40us on 54 serialized ~730ns dma_start
    issues).  p loads issue per-row on the ACT HWDGE ring, everything
    else on the sync ring; the two rings' transfers interleave on the
    16 shared SDMA engines.
  * PE warmup matmuls around phase 0 flip the HAM clock gate
    (1.2 -> 2.4 GHz) before the first score matmul and keep it warm.
  * all four rows' phase-B results accumulate in two shared [128,512]
    PSUM banks at partition 32*b (matmul tile_position from the out
    AP), so the Z-normalization is 2 batched tensor_scalars instead
    of 8, and Z itself is one reduce + one reciprocal over a [128,nch]
    bank the per-row ones-matmuls write at partition 32*b.
  * f split bf16/int8 (int8 dequant on DVE only -- the GPSIMD software
    tensor_scalar path measures ~7.7us per op and poisons concurrent
    DVE ops).
  * host-side prep is layout/dtype only (transposes, gather by mask,
    casts): all arithmetic of the module stays on device.
"""

import sys

import ml_dtypes
import numpy as np

sys.path.insert(0, "/opt/trn_rl_repo")

import concourse.bass as bass  # noqa: E402
import concourse.tile as tile  # noqa: E402
from concourse import bacc, mybir  # noqa: E402
from concourse.bass_utils import run_bass_kernel_spmd  # noqa: E402

N_CORES = 8
B, L, RNN, A = 32, 2048, 1024, 512
BS = B // N_CORES          # 4 batch rows per core
NRC = RNN // 128           # 8 contraction chunks for att_h
NAC = A // 128             # 4 a-chunks (ATT_HID on partitions)
C8 = 512                   # int8 feature columns (tail of RNN)
C16 = RNN - C8             # bf16 feature columns (head of RNN)
N_WARM1 = 60               # PE warmup matmuls before phase 0
N_WARM2 = 130              # gate-pinned warmup matmuls after phase 0

F32 = mybir.dt.float32
FP8 = mybir.dt.float8e4
I8 = mybir.dt.int8
BF16 = mybir.dt.bfloat16
TANH = mybir.ActivationFunctionType.Tanh
EXP = mybir.ActivationFunctionType.Exp

KERNEL_VERSION = 38


def row_cuts(b, nch):
    """f sub-chunk boundaries (lch indices) per row.  Early rows arrive
    long before their phase B -- one chunk.  Late rows are chunked so
    phase-B matmuls start before the full row lands."""
    if b == 0 or b == 1:
        cuts = [0, nch]
    elif b == 2:
        cuts = [0, (nch + 1) // 2, nch]
    else:
        cuts = sorted({0, max(1, nch - 6), max(2, nch - 4), nch - 1, nch})
    return cuts


def build_program(nch, bs=BS, rnn=RNN, a=A):
    lc = nch * 128
    nblob = 4 + bs * nch + 2 + KERNEL_VERSION  # bh | fs | wa(bitcast) | pad
    nc = bacc.Bacc(None, target_bir_lowering=False)
    # p[q, b, ac, l] = p_padded[b, l, ac*128+q]   (A on partitions)
    p = nc.dram_tensor("p", [128, bs, NAC, lc], FP8, kind="ExternalInput")
    # f16[q, b, n, r] = f_padded[b, n*128+q, r] for r < C16 (bf16);
    # f8i holds r >= C16 as int8 with per-position scale (0 for pads)
    f16 = nc.dram_tensor("f16", [128, bs, nch, C16], BF16,
                         kind="ExternalInput")
    f8i = nc.dram_tensor("f8i", [128, bs, nch, C8], I8, kind="ExternalInput")
    # ALL small weights in ONE fp8 blob with large contiguous
    # per-partition descriptors -- a transfer whose per-partition run is
    # tiny (e.g. 32B) head-of-line-blocks every SDMA engine for ~6us.
    #   cols [0:4096)            wh[q, ac*1024+rc*128+j] = W_h[ac*128+j,
    #                            rc*128+q]   (ac-major)
    #   cols [4096:4128)         h[q, rc*bs+b] = h[b, rc*128+q]
    #   cols [4128:4128+4*nblob) f32 blob bitcast: b_h | fs | W_a | pad
    #                            (pad encodes KERNEL_VERSION: the compile
    #                            cache keys on shapes, not embedded BIR)
    wball = nc.dram_tensor("wball", [128, 4096 + NRC * bs + 4 * nblob],
                           FP8, kind="ExternalInput")
    out = nc.dram_tensor("out", [bs, rnn], F32, kind="ExternalOutput")
    fs0 = 4                    # fs column offset in wf32
    wa0 = 4 + bs * nch         # wa column offset in wf32

    with tile.TileContext(nc) as tc:
        with (
            tc.tile_pool(name="singles", bufs=1) as singles,
            tc.tile_pool(name="ppool", bufs=bs) as ppool,
            tc.tile_pool(name="thpool", bufs=bs) as thpool,
            tc.tile_pool(name="fpool", bufs=bs) as fpool,
            tc.tile_pool(name="sm", bufs=4) as smpool,
            tc.tile_pool(name="respool", bufs=1) as respool,
            tc.tile_pool(name="ps_sc", bufs=2, space="PSUM") as ps_sc,
            tc.tile_pool(name="psacc", bufs=1, space="PSUM") as psacc,
            tc.tile_pool(name="pssmall", bufs=1, space="PSUM") as pssmall,
        ):
            # ---- ALL loads go on the sync HWDGE ring in deadline
            # order (EDF): issues (~0.65us each) pipeline far ahead of
            # the ~28us transfer stream, so a single ring sustains full
            # bandwidth AND keeps the ACT ring free for the tanh chain
            # (a dma_start on the ACT ring costs its queue ~0.6-3us).
            wball_sb = singles.tile([128, 4096 + NRC * bs + 4 * nblob], FP8)
            nc.sync.dma_start(out=wball_sb, in_=wball[:, :])
            ptiles = []
            for b in range(bs):
                ptiles.append(ppool.tile([128, NAC, lc], FP8, tag="p",
                                         name=f"pt{b}"))
            nc.sync.dma_start(out=ptiles[0][:, 0:2, :], in_=p[:, 0, 0:2, :])
            nc.sync.dma_start(out=ptiles[0][:, 2:4, :], in_=p[:, 0, 2:4, :])
            nc.sync.dma_start(out=ptiles[1], in_=p[:, 1, :, :])
            h0 = 4096
            blob = wball_sb[:, h0 + NRC * bs:].bitcast(F32)   # [128, nblob]
            wa_ap = wball_sb[:, h0 + NRC * bs + 4 * wa0:
                             h0 + NRC * bs + 4 * wa0 + 8].bitcast(BF16)

            # ---- constants; ACT function-table warm while DMAs stream
            ones_sb = singles.tile([128, 1], BF16)
            nc.vector.memset(ones_sb, 1.0)
            warm16 = singles.tile([128, 64], BF16)
            nc.gpsimd.memset(warm16, 0.5)
            attb = singles.tile([128, NAC, bs], F32)
            warm_sb = singles.tile([128, 1], BF16)
            nc.scalar.activation(out=warm_sb, in_=ones_sb, func=TANH)

            # ---- f loads (sync ring), row-major, int8 before bf16 so
            # the dequant chain starts as early as possible per row.
            f16tiles, fitiles, fdtiles = [], [], []
            for b in range(bs):
                f16tiles.append(fpool.tile([128, nch, C16], BF16,
                                           tag="f16", name=f"ft{b}"))
                fitiles.append(fpool.tile([128, nch, C8], I8,
                                          tag="fi", name=f"fi{b}"))
                fdtiles.append(fpool.tile([128, nch, C8], BF16,
                                          tag="fd", name=f"fd{b}"))
            for b in range(bs):
                if b >= 2:
                    nc.sync.dma_start(out=ptiles[b], in_=p[:, b, :, :])
                cuts = row_cuts(b, nch)
                for k in range(len(cuts) - 1):
                    c0, c1 = cuts[k], cuts[k + 1]
                    nc.sync.dma_start(out=fitiles[b][:, c0:c1, :],
                                      in_=f8i[:, b, c0:c1, :])
                    nc.sync.dma_start(out=f16tiles[b][:, c0:c1, :],
                                      in_=f16[:, b, c0:c1, :])

            # ---- PE warmup: dependency-free tiny matmuls from t~0 keep
            # the PE busy so the HAM clock gate opens (1.2 -> 2.4 GHz)
            # before real matmuls run.  Phase 0 is sandwiched between
            # the two warmup bursts: burst 1 spans the wb8 transfer,
            # burst 2 spans row 0's tanh (PE otherwise idle there).
            # one PSUM bank shared by the warmup sink (cols 16:80,
            # partition 0) and the phase-0 accumulator (cols 0:16) --
            # disjoint regions, sequential groups.
            with tc.tile_pool(name="boot", bufs=1, space="PSUM") as boot:
                bt = boot.tile([128, 80], F32, tag="boot")
                wps = bt[0:1, 16:80]
                for _ in range(N_WARM1):
                    nc.tensor.matmul(wps, lhsT=warm16[:, 0:1], rhs=warm16,
                                     start=True, stop=True,
                                     skip_group_check=True)

                # phase 0: attb[:, ac, b] = (W_h @ h[b] + b_h),
                # a-on-partitions; bs as stream dim; per-ac so each
                # attb chunk unblocks its tanh as its W_h chunk lands.
                for ac in range(NAC):
                    for rc in range(NRC):
                        nc.tensor.matmul(
                            bt[:, ac * bs:(ac + 1) * bs],
                            lhsT=wball_sb[:, ac * 1024 + rc * 128:
                                          ac * 1024 + rc * 128 + 128],
                            rhs=wball_sb[:, h0 + rc * bs:
                                         h0 + (rc + 1) * bs],
                            start=(rc == 0), stop=(rc == NRC - 1),
                            skip_group_check=True)
                    nc.vector.tensor_scalar_add(
                        attb[:, ac, :], bt[:, ac * bs:(ac + 1) * bs],
                        blob[:, ac:ac + 1])
                    if ac == 0:
                        gate = singles.tile([128, 64], BF16)
                        nc.vector.tensor_scalar_mul(gate, warm16,
                                                    attb[:, 0, 0:1])

                # gate-pinned warmup: rhs is written on DVE after the
                # first attb add, so the scheduler cannot hoist these
                # ahead of phase 0.  They run as one dependency-free
                # block filling the PE window between phase 0 and the
                # first tanh-gated score matmuls -- sparse PE activity
                # there re-throttles the HAM clock gate (1.2 GHz) and
                # the cold window then cascades into multi-us EXP
                # stalls on the serial ACT chain.
                for _ in range(N_WARM2):
                    nc.tensor.matmul(wps, lhsT=warm16[:, 0:1],
                                     rhs=gate, start=True, stop=True,
                                     skip_group_check=True)

            # shared result/Z banks: row b lives at partition 64*(b%2)
            # of bank-pair b//2 (matmul tile_position inferred from the
            # out AP; base partitions are limited to {0, 32, 64}).
            z_ps = pssmall.tile([128, 2, nch], F32, tag="zps")
            rps = [[psacc.tile([128, 512], F32, tag=f"r{j}{hh}",
                               name=f"r{j}{hh}")
                    for hh in range(2)] for j in range(2)]

            # ---- pass 1 (phase A for every row)
            w_sbs = []
            for b in range(bs):
                ptile = ptiles[b]
                # tanh with fused per-partition bias (fp8 in, bf16 out)
                th = thpool.tile([128, NAC, lc], BF16, tag="th",
                                 name=f"th{b}")
                for ac in range(NAC):
                    nc.scalar.activation(
                        out=th[:, ac, :], in_=ptile[:, ac, :],
                        func=TANH, bias=attb[:, ac, b:b + 1])
                # scores: lch-outer so each PSUM column's accumulation
                # group is issued contiguously (interleaved groups in one
                # bank corrupt accumulation on HW)
                sc_ps = ps_sc.tile([128, nch], F32, tag="sc", name=f"sc{b}")
                for lch in range(nch):
                    for ac in range(NAC):
                        nc.tensor.matmul(
                            sc_ps[:, lch:lch + 1],
                            lhsT=th[:, ac, lch * 128:(lch + 1) * 128],
                            rhs=wa_ap[:, ac:ac + 1],
                            start=(ac == 0), stop=(ac == NAC - 1))

                # softmax weights (no max subtraction needed: |scores| <=
                # sum|W_a| ~ 11.3, exp cannot overflow f32).  Z partial:
                # ones-matmul row-sum into partition 32*b of the shared
                # Z bank.
                w_sb = smpool.tile([128, nch], BF16, tag="w", name=f"w{b}")
                nc.scalar.activation(out=w_sb, in_=sc_ps, func=EXP)
                zrow = z_ps[64 * (b % 2):64 * (b % 2) + 1, b // 2, :]
                nc.tensor.matmul(zrow, lhsT=ones_sb, rhs=w_sb,
                                 start=True, stop=True,
                                 skip_group_check=True)
                w_sbs.append(w_sb)

            # ---- pass 2 (phase B): row b accumulates w_b @ f_b into
            # partition 32*b of the two shared half-banks.
            def emit_dq(b):
                # dequantize the int8 half on DVE: per-position scale is
                # a per-partition scalar per lch
                fi, fd = fitiles[b], fdtiles[b]
                for lch in range(nch):
                    nc.vector.tensor_scalar_mul(
                        fd[:, lch, :], fi[:, lch, :],
                        blob[:, fs0 + b * nch + lch:fs0 + b * nch + lch + 1])

            def finalize_pair(j):
                # Z and normalization, batched across the pair's rows:
                # one reduce, one reciprocal, two scales, two stores.
                # Garbage at partitions not in {0, 64} is never read.
                zsum = smpool.tile([128, 1], F32, tag="zsum", name=f"zs{j}")
                nc.vector.tensor_reduce(zsum, z_ps[:, j, :],
                                        mybir.AxisListType.X,
                                        mybir.AluOpType.add)
                zinv = smpool.tile([128, 1], F32, tag="zinv", name=f"zi{j}")
                nc.vector.reciprocal(zinv, zsum)
                res = respool.tile([128, rnn], F32, tag=f"res{j}")
                for hh in range(2):
                    nc.vector.tensor_scalar_mul(
                        res[:, hh * 512:(hh + 1) * 512], rps[j][hh], zinv)
                for k in range(2):
                    ring = nc.sync if k == 0 else nc.scalar
                    ring.dma_start(out=out[2 * j + k:2 * j + k + 1, :],
                                   in_=res[64 * k:64 * k + 1, :])

            emit_dq(0)
            emit_dq(1)
            for b in range(bs):
                w_sb = w_sbs[b]
                # bf16 half first (no dequant dependency) except the last
                # row, whose int8 chunks land first in the DMA stream.
                halves = ((0, f16tiles[b]), (1, fdtiles[b]))
                if b == bs - 1:
                    halves = (halves[1], halves[0])
                first = True
                for hh, tile_src in halves:
                    dst = rps[b // 2][hh][64 * (b % 2):64 * (b % 2) + 1, :]
                    for lch in range(nch):
                        nc.tensor.matmul(
                            dst, lhsT=w_sb[:, lch:lch + 1],
                            rhs=tile_src[:, lch, :],
                            start=(lch == 0), stop=(lch == nch - 1),
                            skip_group_check=True)
                    if first and b + 2 < bs:
                        emit_dq(b + 2)
                        first = False
                if b % 2 == 1:
                    finalize_pair(b // 2)
    nc.finalize()
    return nc


_PROGS = {}


def _get_program(nch):
    if nch not in _PROGS:
        _PROGS[nch] = build_program(nch)
    return _PROGS[nch]


def make_in_maps(h, att_feats, p_att_feats, mask, W_h, b_h, W_a):
    h = np.asarray(h, dtype=np.float32)
    att_feats = np.asarray(att_feats, dtype=np.float32)
    p_att_feats = np.asarray(p_att_feats, dtype=np.float32)
    mask = np.asarray(mask)
    W_h = np.asarray(W_h, np.float32)
    b_h = np.asarray(b_h, np.float32).reshape(A)
    wa = np.asarray(W_a, np.float32).reshape(A)

    keep = ~mask                                   # [B, L] kept positions
    cnts = keep.sum(axis=1)
    nch = max(1, -(-int(cnts.max()) // 128))
    lc = nch * 128

    # padding p-row: tanh saturates to -sign(wa) => score = -sum|wa|
    p_pad = np.where(wa >= 0.0, -20.0, 20.0).astype(ml_dtypes.float8_e4m3)

    p2 = np.empty((B, 128, NAC, lc), dtype=ml_dtypes.float8_e4m3)
    f16h = np.zeros((B, 128, nch, C16), dtype=ml_dtypes.bfloat16)
    f8ih = np.zeros((B, 128, nch, C8), dtype=np.int8)
    fsc = np.zeros((B, lc), dtype=np.float32)
    for b in range(B):
        idx = np.flatnonzero(keep[b])
        n = idx.size
        pb = np.empty((lc, A), dtype=ml_dtypes.float8_e4m3)
        pb[:n] = p_att_feats[b, idx]
        pb[n:] = p_pad
        # [lc, A] -> [128(q), NAC, lc]
        p2[b] = pb.reshape(lc, NAC, 128).transpose(2, 1, 0)
        fb = att_feats[b, idx]                          # [n, RNN]
        lo = np.zeros((lc, C16), dtype=ml_dtypes.bfloat16)
        lo[:n] = fb[:, :C16].astype(ml_dtypes.bfloat16)
        f16h[b] = lo.reshape(nch, 128, C16).transpose(1, 0, 2)
        # int8 half with per-position scale (exact 0 for pads)
        hi = fb[:, C16:]
        amax = np.abs(hi).max(axis=1)
        scale = amax / 127.0
        q = np.rint(hi / scale[:, None]).clip(-127, 127).astype(np.int8)
        qpad = np.zeros((lc, C8), dtype=np.int8)
        qpad[:n] = q
        f8ih[b] = qpad.reshape(nch, 128, C8).transpose(1, 0, 2)
        fsc[b, :n] = scale

    # wh8[q, ac*1024+rc*128+j] = W_h[ac*128+j, rc*128+q]
    wh8 = np.ascontiguousarray(
        W_h.reshape(NAC, 128, NRC, 128).transpose(3, 0, 2, 1)).astype(
            ml_dtypes.float8_e4m3).reshape(128, 4096)
    bh2 = np.ascontiguousarray(b_h.reshape(NAC, 128).T)  # [128, NAC] f32
    wa2 = np.ascontiguousarray(wa.reshape(NAC, 128).T).astype(
        ml_dtypes.bfloat16)                             # [128, NAC]
    wa_f32view = np.ascontiguousarray(wa2).view(np.float32)  # [128, 2]

    nblob = 4 + BS * nch + 2 + KERNEL_VERSION
    in_maps = []
    for c in range(N_CORES):
        s = slice(c * BS, (c + 1) * BS)
        h2c = np.ascontiguousarray(
            h[s].reshape(BS, NRC, 128).transpose(2, 1, 0)).astype(
                ml_dtypes.float8_e4m3).reshape(128, NRC * BS)
        # fs[q, b, n]: scale for position l = n*128+q of row b
        fsc_c = np.ascontiguousarray(
            fsc[s].reshape(BS, nch, 128).transpose(2, 0, 1))
        wf32 = np.zeros((128, nblob), dtype=np.float32)
        wf32[:, 0:4] = bh2
        wf32[:, 4:4 + BS * nch] = fsc_c.reshape(128, BS * nch)
        wf32[:, 4 + BS * nch:4 + BS * nch + 2] = wa_f32view
        wball = np.empty((128, 4096 + NRC * BS + 4 * nblob),
                         dtype=ml_dtypes.float8_e4m3)
        wball[:, 0:4096] = wh8
        wball[:, 4096:4096 + NRC * BS] = h2c
        wball[:, 4096 + NRC * BS:] = wf32.view(np.uint8).view(
            ml_dtypes.float8_e4m3)
        in_maps.append({
            "p": np.ascontiguousarray(p2[s].transpose(1, 0, 2, 3)),
            "f16": np.ascontiguousarray(f16h[s].transpose(1, 0, 2, 3)),
            "f8i": np.ascontiguousarray(f8ih[s].transpose(1, 0, 2, 3)),
            "wball": wball,
        })
    return in_maps, nch


def run_sharded(inputs, trace=False, **kwargs):
    in_maps, nch = make_in_maps(
        inputs["h"], inputs["att_feats"], inputs["p_att_feats"],
        inputs["mask"], inputs["W_h"], inputs["b_h"], inputs["W_a"])
    nc = _get_program(nch)
    return run_bass_kernel_spmd(nc, in_maps, core_ids=list(range(N_CORES)),
                                trace=trace, **kwargs)


def kernel(h, att_feats, p_att_feats, mask, W_h, b_h, W_a, b_a):
    res = run_sharded({
        "h": h, "att_feats": att_feats, "p_att_feats": p_att_feats,
        "mask": mask, "W_h": W_h, "b_h": b_h, "W_a": W_a, "b_a": b_a})
    return np.concatenate([res.results[c]["out"] for c in range(N_CORES)],
                          axis=0).astype(np.float32)
